# revision 1
# baseline (speedup 1.0000x reference)
"""PinSAGE-style sampled-neighbor mean + linear on 8 Trainium2 NeuronCores.

Strategy (per sharding hint): shard the 100k nodes across the 8 cores
(12.5k nodes each, padded to 12800 = 100 groups of 128); x is replicated.
Each core gathers its nodes' (up to) 10 sampled neighbor rows from HBM with
indirect DMA (one 128-row gather per node-group tap column), computes the
neighbor mean via a transpose-accumulate on the PE plus a per-partition
correction (padding slots duplicate tap 0), applies the 128x128 linear on
the PE, and streams the result out.

Host-side prep only builds index/weight tables (the "sampling" step):
first-10-edges-per-node selection, padding, and per-core layouts.
"""

import numpy as np

N_NODES = 100000
N_EDGES = 1600000
D = 128
TAPS = 10
N_CORES = 8
NODES_PER_CORE = 12500
GROUPS = 100                      # 100 groups of 128 = 12800 padded nodes
NODES_PAD = GROUPS * 128
IDX_CHUNK = 10                    # groups per idx/wmeta DMA chunk

_cache = {}


def _build_tables(edge_index):
    """First-TAPS-edges-per-node neighbor table with torch-masking semantics.

    Returns idx [N_NODES, TAPS] int32 (padding slots = copy of slot 0),
    w1mk [N_NODES] f32 = (c' - 9), inv [N_NODES] f32 = 1/c'.
    """
    row = np.asarray(edge_index[0], dtype=np.int64)
    col = np.asarray(edge_index[1], dtype=np.int64)
    E = row.shape[0]
    order = np.argsort(row, kind="stable")
    row_s = row[order]
    col_s = col[order]
    starts = np.searchsorted(row_s, np.arange(N_NODES, dtype=np.int64))
    counts = np.diff(np.append(starts, E))
    rank = np.arange(E, dtype=np.int64) - starts[row_s]
    keep = rank < TAPS
    kr = row_s[keep]
    kc = col_s[keep]
    krank = rank[keep]

    idx = np.zeros((N_NODES, TAPS), np.int64)
    idx[kr, krank] = kc
    cnt = np.minimum(counts, TAPS)
    # fallback: no out-edges -> single self tap
    self_nodes = cnt == 0
    idx[self_nodes, 0] = np.nonzero(self_nodes)[0]
    cnt_eff = np.maximum(cnt, 1)
    # pad slots j >= cnt_eff with a copy of slot 0
    cols = np.arange(TAPS)[None, :]
    pad = cols >= cnt_eff[:, None]
    first = idx[:, 0]
    idx = np.where(pad, first[:, None], idx)
    w1mk = (cnt_eff - 9).astype(np.float32)
    inv = (1.0 / cnt_eff).astype(np.float32)
    return idx.astype(np.int32), w1mk, inv


def _build_program():
    import concourse.bass as bass
    import concourse.mybir as mybir
    import concourse.tile as tile
    from concourse import bacc
    from concourse.masks import make_identity

    nc = bacc.Bacc("TRN2", target_bir_lowering=False, debug=False,
                   enable_asserts=True, num_devices=N_CORES)
    x = nc.dram_tensor("x", [N_NODES, D], mybir.dt.float32,
                       kind="ExternalInput").ap()
    idx = nc.dram_tensor("idx", [GROUPS // IDX_CHUNK, 128, IDX_CHUNK * TAPS],
                         mybir.dt.int32, kind="ExternalInput").ap()
    wmeta = nc.dram_tensor("wmeta", [GROUPS // IDX_CHUNK, 128, IDX_CHUNK * 2],
                           mybir.dt.float32, kind="ExternalInput").ap()
    wt = nc.dram_tensor("wt", [D, D], mybir.dt.float32,
                        kind="ExternalInput").ap()
    bias_rep = nc.dram_tensor("bias_rep", [128, D], mybir.dt.float32,
                              kind="ExternalInput").ap()
    out = nc.dram_tensor("out", [NODES_PAD, D], mybir.dt.float32,
                         kind="ExternalOutput").ap()

    with tile.TileContext(nc) as tc:
        with tc.tile_pool(name="const", bufs=1) as const_p, \
             tc.tile_pool(name="meta", bufs=2) as meta_p, \
             tc.tile_pool(name="gat", bufs=4) as gat_p, \
             tc.tile_pool(name="cmb", bufs=3) as cmb_p, \
             tc.tile_pool(name="outp", bufs=3) as out_p, \
             tc.tile_pool(name="ps1", bufs=2, space="PSUM") as ps1_p, \
             tc.tile_pool(name="ps2", bufs=2, space="PSUM") as ps2_p:

            ident = const_p.tile([128, 128], mybir.dt.float32)
            make_identity(nc, ident[:])
            wt_sb = const_p.tile([D, D], mybir.dt.float32)
            nc.sync.dma_start(wt_sb[:], wt[:])
            bias_sb = const_p.tile([128, D], mybir.dt.float32)
            nc.sync.dma_start(bias_sb[:], bias_rep[:])

            for t in range(GROUPS // IDX_CHUNK):
                idx_t = meta_p.tile([128, IDX_CHUNK * TAPS], mybir.dt.int32,
                                    name="idx_t")
                wm_t = meta_p.tile([128, IDX_CHUNK * 2], mybir.dt.float32,
                                   name="wm_t")
                nc.sync.dma_start(idx_t[:], idx[t])
                nc.sync.dma_start(wm_t[:], wmeta[t])
                for gl in range(IDX_CHUNK):
                    g = t * IDX_CHUNK + gl
                    G = gat_p.tile([128, TAPS * D], mybir.dt.float32, name="G")
                    for j in range(TAPS):
                        nc.gpsimd.indirect_dma_start(
                            out=G[:, j * D:(j + 1) * D],
                            out_offset=None,
                            in_=x[:],
                            in_offset=bass.IndirectOffsetOnAxis(
                                ap=idx_t[:, gl * TAPS + j: gl * TAPS + j + 1],
                                axis=0),
                        )
                    # C0 = G0*(c'-9) + G1 ; C[1..4] = (G2+G6, G3+G7, G4+G8, G5+G9)
                    C = cmb_p.tile([128, 5 * D], mybir.dt.float32, name="C")
                    nc.vector.scalar_tensor_tensor(
                        out=C[:, 0:D],
                        in0=G[:, 0:D],
                        scalar=wm_t[:, 2 * gl: 2 * gl + 1],
                        in1=G[:, D:2 * D],
                        op0=mybir.AluOpType.mult,
                        op1=mybir.AluOpType.add,
                    )
                    nc.vector.tensor_tensor(
                        out=C[:, D:5 * D],
                        in0=G[:, 2 * D:6 * D],
                        in1=G[:, 6 * D:10 * D],
                        op=mybir.AluOpType.add,
                    )
                    # psum1[d, n] = sum_i C_i.T  (transpose-accumulate on PE)
                    psum1 = ps1_p.tile([128, 128], mybir.dt.float32,
                                       space="PSUM", name="psum1")
                    for i in range(5):
                        nc.tensor.matmul(
                            psum1[:],
                            lhsT=C[:, i * D:(i + 1) * D],
                            rhs=ident[:],
                            is_transpose=True,
                            start=(i == 0),
                            stop=(i == 4),
                        )
                    sT = cmb_p.tile([128, 128], mybir.dt.float32, name="sT")
                    nc.scalar.copy(sT[:], psum1[:])
                    # psum2[n, dout] = sum_d sT[d, n] * Wt[d, dout]
                    psum2 = ps2_p.tile([128, 128], mybir.dt.float32,
                                       space="PSUM", name="psum2")
                    nc.tensor.matmul(psum2[:], lhsT=sT[:], rhs=wt_sb[:],
                                     start=True, stop=True)
                    o_sb = out_p.tile([128, D], mybir.dt.float32, name="o_sb")
                    nc.vector.scalar_tensor_tensor(
                        out=o_sb[:],
                        in0=psum2[:],
                        scalar=wm_t[:, 2 * gl + 1: 2 * gl + 2],
                        in1=bias_sb[:],
                        op0=mybir.AluOpType.mult,
                        op1=mybir.AluOpType.add,
                    )
                    nc.sync.dma_start(out[g * 128:(g + 1) * 128, :], o_sb[:])
    nc.compile()
    return nc


def kernel(x, edge_index, W, b):
    from concourse.bass_utils import run_bass_kernel_spmd

    x = np.ascontiguousarray(np.asarray(x, dtype=np.float32))
    W = np.asarray(W, dtype=np.float32)
    b = np.asarray(b, dtype=np.float32)

    idx_all, w1mk_all, inv_all = _build_tables(edge_index)

    # per-core tables, padded to NODES_PAD nodes
    in_maps = []
    wt_host = np.ascontiguousarray(W.T)
    bias_host = np.ascontiguousarray(np.broadcast_to(b[None, :], (128, D)))
    for c in range(N_CORES):
        lo = c * NODES_PER_CORE
        hi = lo + NODES_PER_CORE
        idx_c = np.zeros((NODES_PAD, TAPS), np.int32)
        idx_c[:NODES_PER_CORE] = idx_all[lo:hi]
        w1_c = np.full(NODES_PAD, 1.0, np.float32)
        w1_c[:NODES_PER_CORE] = w1mk_all[lo:hi]
        # pad nodes: all taps point at row 0, c'=1 semantics -> w1mk = -8, inv = 1
        w1_c[NODES_PER_CORE:] = -8.0
        iv_c = np.ones(NODES_PAD, np.float32)
        iv_c[:NODES_PER_CORE] = inv_all[lo:hi]
        # layout: idx [GROUPS/IDX_CHUNK, 128, IDX_CHUNK*TAPS]
        idx_g = idx_c.reshape(GROUPS // IDX_CHUNK, IDX_CHUNK, 128, TAPS)
        idx_dram = np.ascontiguousarray(
            idx_g.transpose(0, 2, 1, 3).reshape(GROUPS // IDX_CHUNK, 128,
                                                IDX_CHUNK * TAPS))
        wm = np.stack([w1_c, iv_c], axis=1)  # [NODES_PAD, 2]
        wm_g = wm.reshape(GROUPS // IDX_CHUNK, IDX_CHUNK, 128, 2)
        wm_dram = np.ascontiguousarray(
            wm_g.transpose(0, 2, 1, 3).reshape(GROUPS // IDX_CHUNK, 128,
                                               IDX_CHUNK * 2))
        in_maps.append({
            "x": x,
            "idx": idx_dram,
            "wmeta": wm_dram,
            "wt": wt_host,
            "bias_rep": bias_host,
        })

    if "nc" not in _cache:
        _cache["nc"] = _build_program()
    nc = _cache["nc"]

    res = run_bass_kernel_spmd(nc, in_maps, core_ids=list(range(N_CORES)))
    outs = [res.results[c]["out"][:NODES_PER_CORE] for c in range(N_CORES)]
    return np.concatenate(outs, axis=0)


# revision 2
# speedup vs baseline: 1.0071x; 1.0071x over previous
"""PinSAGE-style sampled-neighbor mean + linear on 8 Trainium2 NeuronCores.

Strategy (per sharding hint): shard the 100k nodes across the 8 cores
(12.5k nodes each, padded to 12800 = 100 groups of 128); x is replicated.
Each core gathers its nodes' (up to) 10 sampled neighbor rows from HBM with
indirect DMA (one 128-row gather per node-group tap column), computes the
neighbor mean via a transpose-accumulate on the PE plus a per-partition
correction (padding slots duplicate tap 0), applies the 128x128 linear on
the PE, and streams the result out.

Host-side prep only builds index/weight tables (the "sampling" step):
first-10-edges-per-node selection, padding, and per-core layouts.
"""

import numpy as np

N_NODES = 100000
N_EDGES = 1600000
D = 128
TAPS = 10
N_CORES = 8
NODES_PER_CORE = 12500
GROUPS = 100                      # 100 groups of 128 = 12800 padded nodes
NODES_PAD = GROUPS * 128
IDX_CHUNK = 10                    # groups per idx/wmeta DMA chunk

_cache = {}


def _build_tables(edge_index):
    """First-TAPS-edges-per-node neighbor table with torch-masking semantics.

    Returns idx [N_NODES, TAPS] int32 (padding slots = copy of slot 0),
    w1mk [N_NODES] f32 = (c' - 9), inv [N_NODES] f32 = 1/c'.
    """
    row = np.asarray(edge_index[0], dtype=np.int64)
    col = np.asarray(edge_index[1], dtype=np.int64)
    E = row.shape[0]
    order = np.argsort(row, kind="stable")
    row_s = row[order]
    col_s = col[order]
    starts = np.searchsorted(row_s, np.arange(N_NODES, dtype=np.int64))
    counts = np.diff(np.append(starts, E))
    rank = np.arange(E, dtype=np.int64) - starts[row_s]
    keep = rank < TAPS
    kr = row_s[keep]
    kc = col_s[keep]
    krank = rank[keep]

    idx = np.zeros((N_NODES, TAPS), np.int64)
    idx[kr, krank] = kc
    cnt = np.minimum(counts, TAPS)
    # fallback: no out-edges -> single self tap
    self_nodes = cnt == 0
    idx[self_nodes, 0] = np.nonzero(self_nodes)[0]
    cnt_eff = np.maximum(cnt, 1)
    # pad slots j >= cnt_eff with a copy of slot 0
    cols = np.arange(TAPS)[None, :]
    pad = cols >= cnt_eff[:, None]
    first = idx[:, 0]
    idx = np.where(pad, first[:, None], idx)
    w1mk = (cnt_eff - 9).astype(np.float32)
    inv = (1.0 / cnt_eff).astype(np.float32)
    return idx.astype(np.int32), w1mk, inv


def _build_program():
    import concourse.bass as bass
    import concourse.mybir as mybir
    import concourse.tile as tile
    from concourse import bacc
    from concourse.masks import make_identity

    nc = bacc.Bacc("TRN2", target_bir_lowering=False, debug=False,
                   enable_asserts=True, num_devices=N_CORES)
    x = nc.dram_tensor("x", [N_NODES, D], mybir.dt.float32,
                       kind="ExternalInput").ap()
    idx = nc.dram_tensor("idx", [GROUPS // IDX_CHUNK, 128, IDX_CHUNK * TAPS],
                         mybir.dt.int32, kind="ExternalInput").ap()
    wmeta = nc.dram_tensor("wmeta", [GROUPS // IDX_CHUNK, 128, IDX_CHUNK * 2],
                           mybir.dt.float32, kind="ExternalInput").ap()
    wt = nc.dram_tensor("wt", [D, D], mybir.dt.float32,
                        kind="ExternalInput").ap()
    bias_rep = nc.dram_tensor("bias_rep", [128, D], mybir.dt.float32,
                              kind="ExternalInput").ap()
    out = nc.dram_tensor("out", [NODES_PAD, D], mybir.dt.float32,
                         kind="ExternalOutput").ap()

    with tile.TileContext(nc) as tc:
        with tc.tile_pool(name="const", bufs=1) as const_p, \
             tc.tile_pool(name="meta", bufs=3) as meta_p, \
             tc.tile_pool(name="gat", bufs=8) as gat_p, \
             tc.tile_pool(name="cmb", bufs=4) as cmb_p, \
             tc.tile_pool(name="outp", bufs=4) as out_p, \
             tc.tile_pool(name="ps1", bufs=2, space="PSUM") as ps1_p, \
             tc.tile_pool(name="ps2", bufs=2, space="PSUM") as ps2_p:

            ident = const_p.tile([128, 128], mybir.dt.float32)
            make_identity(nc, ident[:])
            wt_sb = const_p.tile([D, D], mybir.dt.float32)
            nc.sync.dma_start(wt_sb[:], wt[:])
            bias_sb = const_p.tile([128, D], mybir.dt.float32)
            nc.sync.dma_start(bias_sb[:], bias_rep[:])

            for t in range(GROUPS // IDX_CHUNK):
                idx_t = meta_p.tile([128, IDX_CHUNK * TAPS], mybir.dt.int32,
                                    name="idx_t")
                wm_t = meta_p.tile([128, IDX_CHUNK * 2], mybir.dt.float32,
                                   name="wm_t")
                nc.sync.dma_start(idx_t[:], idx[t])
                nc.sync.dma_start(wm_t[:], wmeta[t])
                for gl in range(IDX_CHUNK):
                    g = t * IDX_CHUNK + gl
                    G = gat_p.tile([128, TAPS * D], mybir.dt.float32, name="G")
                    for j in range(TAPS):
                        nc.gpsimd.indirect_dma_start(
                            out=G[:, j * D:(j + 1) * D],
                            out_offset=None,
                            in_=x[:],
                            in_offset=bass.IndirectOffsetOnAxis(
                                ap=idx_t[:, gl * TAPS + j: gl * TAPS + j + 1],
                                axis=0),
                        )
                    # C0 = G0*(c'-9) + G1 ; C[1..4] = (G2+G6, G3+G7, G4+G8, G5+G9)
                    C = cmb_p.tile([128, 5 * D], mybir.dt.float32, name="C")
                    nc.vector.scalar_tensor_tensor(
                        out=C[:, 0:D],
                        in0=G[:, 0:D],
                        scalar=wm_t[:, 2 * gl: 2 * gl + 1],
                        in1=G[:, D:2 * D],
                        op0=mybir.AluOpType.mult,
                        op1=mybir.AluOpType.add,
                    )
                    nc.vector.tensor_tensor(
                        out=C[:, D:5 * D],
                        in0=G[:, 2 * D:6 * D],
                        in1=G[:, 6 * D:10 * D],
                        op=mybir.AluOpType.add,
                    )
                    # psum1[d, n] = sum_i C_i.T  (transpose-accumulate on PE)
                    psum1 = ps1_p.tile([128, 128], mybir.dt.float32,
                                       space="PSUM", name="psum1")
                    for i in range(5):
                        nc.tensor.matmul(
                            psum1[:],
                            lhsT=C[:, i * D:(i + 1) * D],
                            rhs=ident[:],
                            is_transpose=True,
                            start=(i == 0),
                            stop=(i == 4),
                        )
                    sT = cmb_p.tile([128, 128], mybir.dt.float32, name="sT")
                    nc.scalar.copy(sT[:], psum1[:])
                    # psum2[n, dout] = sum_d sT[d, n] * Wt[d, dout]
                    psum2 = ps2_p.tile([128, 128], mybir.dt.float32,
                                       space="PSUM", name="psum2")
                    nc.tensor.matmul(psum2[:], lhsT=sT[:], rhs=wt_sb[:],
                                     start=True, stop=True)
                    o_sb = out_p.tile([128, D], mybir.dt.float32, name="o_sb")
                    nc.vector.scalar_tensor_tensor(
                        out=o_sb[:],
                        in0=psum2[:],
                        scalar=wm_t[:, 2 * gl + 1: 2 * gl + 2],
                        in1=bias_sb[:],
                        op0=mybir.AluOpType.mult,
                        op1=mybir.AluOpType.add,
                    )
                    nc.sync.dma_start(out[g * 128:(g + 1) * 128, :], o_sb[:])
    nc.compile()
    return nc


def kernel(x, edge_index, W, b):
    from concourse.bass_utils import run_bass_kernel_spmd

    x = np.ascontiguousarray(np.asarray(x, dtype=np.float32))
    W = np.asarray(W, dtype=np.float32)
    b = np.asarray(b, dtype=np.float32)

    idx_all, w1mk_all, inv_all = _build_tables(edge_index)

    # per-core tables, padded to NODES_PAD nodes
    in_maps = []
    wt_host = np.ascontiguousarray(W.T)
    bias_host = np.ascontiguousarray(np.broadcast_to(b[None, :], (128, D)))
    for c in range(N_CORES):
        lo = c * NODES_PER_CORE
        hi = lo + NODES_PER_CORE
        idx_c = np.zeros((NODES_PAD, TAPS), np.int32)
        idx_c[:NODES_PER_CORE] = idx_all[lo:hi]
        w1_c = np.full(NODES_PAD, 1.0, np.float32)
        w1_c[:NODES_PER_CORE] = w1mk_all[lo:hi]
        # pad nodes: all taps point at row 0, c'=1 semantics -> w1mk = -8, inv = 1
        w1_c[NODES_PER_CORE:] = -8.0
        iv_c = np.ones(NODES_PAD, np.float32)
        iv_c[:NODES_PER_CORE] = inv_all[lo:hi]
        # layout: idx [GROUPS/IDX_CHUNK, 128, IDX_CHUNK*TAPS]
        idx_g = idx_c.reshape(GROUPS // IDX_CHUNK, IDX_CHUNK, 128, TAPS)
        idx_dram = np.ascontiguousarray(
            idx_g.transpose(0, 2, 1, 3).reshape(GROUPS // IDX_CHUNK, 128,
                                                IDX_CHUNK * TAPS))
        wm = np.stack([w1_c, iv_c], axis=1)  # [NODES_PAD, 2]
        wm_g = wm.reshape(GROUPS // IDX_CHUNK, IDX_CHUNK, 128, 2)
        wm_dram = np.ascontiguousarray(
            wm_g.transpose(0, 2, 1, 3).reshape(GROUPS // IDX_CHUNK, 128,
                                               IDX_CHUNK * 2))
        in_maps.append({
            "x": x,
            "idx": idx_dram,
            "wmeta": wm_dram,
            "wt": wt_host,
            "bias_rep": bias_host,
        })

    if "nc" not in _cache:
        _cache["nc"] = _build_program()
    nc = _cache["nc"]

    res = run_bass_kernel_spmd(nc, in_maps, core_ids=list(range(N_CORES)))
    outs = [res.results[c]["out"][:NODES_PER_CORE] for c in range(N_CORES)]
    return np.concatenate(outs, axis=0)
